# revision 20
# baseline (speedup 1.0000x reference)
"""CQAttention (BiDAF-style context-query attention) on 8 TRN2 NeuronCores.

Full shapes: contex [64, 512, 256], question [64, 64, 256],
W_weight [1, 768], W_bias [1] -> out [64, 512, 1024].

Sharding: pure data-parallel over batch, 8 batches per core.

Math notes (per batch, C=[512,256], Q=[64,256], w=[wq|wc|wi]):
  S[i,j] = sum_d C[i,d]*wi[d]*Q[j,d] + C[i].wc + Q[j].wq + b
  S1 = softmax_j(S), S2 = softmax_i(S)
  - b drops out of both softmaxes; s_c drops out of S1; s_q drops out of S2.
  - E1 = exp(s_i + s_q[j]), r1[i] = sum_j E1;  S1 = E1/r1
  - E2 = exp(s_i + s_c[i]), r2[j] = sum_i E2;  S2 = E2/r2
  - A  = S1 @ Q = (E1 @ Q)/r1
  - Bm = (S1 @ S2^T) @ C = S1 @ (S2^T @ C) = (E1 @ C2)/r1, C2 = (E2^T @ C)/r2
  r1/r2 are obtained for free as ones-columns appended to the matmul rhs.
  out = [C | A | C*A | C*Bm]
"""

import numpy as np

B, LC, LQ, D = 64, 512, 64, 256
NCORES = 8
BL = B // NCORES  # batches per core

_NC_CACHE = None


def _build_nc(stage=99):
    """stage gates how much of the kernel is emitted (for hw bisection):
    1: loads+casts+output DMA of C only
    2: +PE transposes (+W broadcast matmul)
    3: +M1T+E1
    4: +M1'+E2
    5: +M3+C2
    6: +M2+A
    7+: full
    """
    import concourse.bass as bass
    import concourse.mybir as mybir
    from concourse import bacc
    from concourse import masks
    from concourse import tile
    from contextlib import ExitStack

    f32 = mybir.dt.float32
    bf16 = mybir.dt.bfloat16
    AF = mybir.ActivationFunctionType
    ALU = mybir.AluOpType
    ts = bass.ts

    nc = bacc.Bacc("TRN2", target_bir_lowering=False, debug=False)
    C_d = nc.dram_tensor("contex", [BL, LC, D], f32, kind="ExternalInput")
    Q_d = nc.dram_tensor("question", [BL, LQ, D], f32, kind="ExternalInput")
    W_d = nc.dram_tensor("W_weight", [1, 3 * D], f32, kind="ExternalInput")
    out_d = nc.dram_tensor("out", [BL, LC, 4 * D], f32, kind="ExternalOutput")

    with tile.TileContext(nc) as tc, ExitStack() as ctx:
        const = ctx.enter_context(tc.tile_pool(name="const", bufs=1))
        sb = ctx.enter_context(tc.tile_pool(name="sb", bufs=3))
        ps_tr = ctx.enter_context(tc.tile_pool(name="ps_tr", bufs=2, space="PSUM"))
        ps_si = ctx.enter_context(tc.tile_pool(name="ps_si", bufs=2, space="PSUM"))
        ps_mm = ctx.enter_context(tc.tile_pool(name="ps_mm", bufs=3, space="PSUM"))

        # ---- constants ----
        ident = const.tile([128, 128], bf16, tag="ident")
        masks.make_identity(nc, ident[:])

        W_sb = const.tile([1, 3 * D], f32, tag="W_sb")
        nc.sync.dma_start(W_sb[:], W_d[:])

        # wc as two [128,1] columns (k-th contraction tile), fp32
        wc_f32 = const.tile([128, 2, 1], f32, tag="wc_f32")
        nc.sync.dma_start(
            wc_f32[:], W_d[0, D : 2 * D].rearrange("(k p o) -> p k o", p=128, o=1)
        )

        if stage >= 2:
            # broadcast wq/wi rows to 64 partitions via K=1 matmul with ones.
            # Stage the rhs through DVE so the matmul waits on one engine only.
            W_sb2 = const.tile([1, 2, D], f32, tag="W_sb2")
            nc.vector.tensor_copy(W_sb2[:, 0, :], W_sb[0:1, 0:D])
            nc.vector.tensor_copy(W_sb2[:, 1, :], W_sb[0:1, 2 * D : 3 * D])
            ones_row = const.tile([1, LQ], f32, tag="ones_row")
            nc.vector.memset(ones_row[:], 1.0)
            wb_ps = ps_si.tile([LQ, 2, D], f32, tag="si")
            nc.tensor.matmul(wb_ps[:], ones_row[:], W_sb2[:], start=True, stop=True)
            wqi = const.tile([LQ, 2, D], f32, tag="wqi")
            nc.scalar.copy(wqi[:], wb_ps[:])
            wq_b = wqi[:, 0, :]  # [64, 256] rows = wq
            wi_b = wqi[:, 1, :]  # [64, 256] rows = wi

        for b in range(BL):
            # ---- loads ----
            C_f32 = sb.tile([128, 4, D], f32, tag="C_f32")
            nc.sync.dma_start(C_f32[:], C_d[b].rearrange("(t p) d -> p t d", p=128))
            Q_f32 = sb.tile([LQ, D], f32, tag="Q_f32")
            nc.sync.dma_start(Q_f32[:], Q_d[b])

            # ---- bf16 casts / augmented operands ----
            # C_bf: [128, 4, 257]: cols 0:256 = C (bf16), col 256 = 1.0
            C_bf = sb.tile([128, 4, D + 1], bf16, tag="C_bf")
            nc.gpsimd.tensor_copy(C_bf[:, :, 0:D], C_f32[:])
            nc.gpsimd.memset(C_bf[:, :, D : D + 1], 1.0)

            # Q_bf: [64, 257]: cols 0:256 = Q (bf16), col 256 = 1.0
            Q_bf = sb.tile([LQ, D + 1], bf16, tag="Q_bf")
            nc.gpsimd.tensor_copy(Q_bf[:, 0:D], Q_f32[:])
            nc.gpsimd.memset(Q_bf[:, D : D + 1], 1.0)

            if stage < 2 or stage == 21:
                out_r = out_d[b].rearrange("(t p) dd -> p t dd", p=128)
                nc.sync.dma_start(out_r[:, :, 0:D], C_f32[:])
                continue

            # Q' = Q * wi (bf16), s_q = rowsum(Q * wq) (f32)
            # (tensor_tensor_reduce crashes the exec unit on this runtime —
            # use separate mul + reduce instead)
            QP_bf = sb.tile([LQ, D], bf16, tag="QP_bf")
            nc.gpsimd.tensor_mul(QP_bf[:], Q_f32[:], wi_b)
            scr = sb.tile([LQ, D], f32, tag="scr")
            s_q = sb.tile([LQ, 1], f32, tag="s_q")
            nc.vector.tensor_mul(scr[:], Q_f32[:], wq_b)
            nc.vector.reduce_sum(s_q[:], scr[:], axis=mybir.AxisListType.X)

            if stage in (22, 221, 222):
                out_r = out_d[b].rearrange("(t p) dd -> p t dd", p=128)
                nc.sync.dma_start(out_r[:, :, 0:D], C_f32[:])
                continue

            # ---- transposes (PE) ----
            # tq: Q'^T -> [128, 2*64]; QW[k] = [Q'^T_k | wc_k]  [128, 2, 65]
            tq = ps_tr.tile([128, 128], bf16, tag="tr")
            for k in range(2):
                nc.tensor.transpose(
                    tq[:, ts(k, 64)], QP_bf[:, ts(k, 128)], ident[0:LQ, 0:LQ]
                )
            QW = sb.tile([128, 2, 65], bf16, tag="QW")
            nc.vector.tensor_copy(
                QW[:, :, 0:64], tq[:].rearrange("p (k j) -> p k j", k=2)
            )
            nc.vector.tensor_copy(QW[:, :, 64:65], wc_f32[:])

            if stage == 23:
                out_r = out_d[b].rearrange("(t p) dd -> p t dd", p=128)
                nc.sync.dma_start(out_r[:, :, 0:D], C_f32[:])
                continue

            # tc: C^T -> CT [128, 2, 512] (k = d-tile, free = i)
            tcp = ps_tr.tile([128, 2, 512], bf16, tag="tr")
            for t in range(4):
                for k in range(2):
                    nc.tensor.transpose(
                        tcp[:, k, ts(t, 128)],
                        C_bf[:, t, ts(k, 128)],
                        ident[:],
                    )
            CT = sb.tile([128, 2, 512], bf16, tag="CT")
            nc.vector.tensor_copy(CT[:], tcp[:])

            if stage < 3:
                out_r = out_d[b].rearrange("(t p) dd -> p t dd", p=128)
                nc.sync.dma_start(out_r[:, :, 0:D], C_f32[:])
                continue

            # ---- M1T: s_i^T [65, 512] (row 64 = s_c, unused) ----
            si_T = ps_si.tile([65, 512], f32, tag="si")
            for k in range(2):
                nc.tensor.matmul(
                    si_T[:], QW[:, k, :], CT[:, k, :], start=(k == 0), stop=(k == 1)
                )
            # E1_T = exp(s_i^T + s_q) (bf16)  [64, 512]
            E1_T = sb.tile([LQ, 512], bf16, tag="E1_T")
            nc.scalar.activation(E1_T[:], si_T[0:LQ, :], AF.Exp, bias=s_q[:])

            if stage < 4:
                out_r = out_d[b].rearrange("(t p) dd -> p t dd", p=128)
                nc.sync.dma_start(out_r[:, :, 0:D], C_f32[:])
                continue

            # ---- M1': s_i natural [128, 4, 65] (col 64 = s_c) ----
            si_n = ps_si.tile([128, 4, 65], f32, tag="si")
            for t in range(4):
                for k in range(2):
                    nc.tensor.matmul(
                        si_n[:, t, :],
                        CT[:, k, ts(t, 128)],
                        QW[:, k, :],
                        start=(k == 0),
                        stop=(k == 1),
                    )
            sc = sb.tile([128, 4, 1], f32, tag="sc")
            nc.vector.tensor_copy(sc[:], si_n[:, :, 64:65])
            # E2 = exp(s_i + s_c) (bf16)  [128, 4, 64]
            E2 = sb.tile([128, 4, 64], bf16, tag="E2")
            for t in range(4):
                nc.scalar.activation(
                    E2[:, t, :], si_n[:, t, 0:64], AF.Exp, bias=sc[:, t, :]
                )

            if stage < 5:
                out_r = out_d[b].rearrange("(t p) dd -> p t dd", p=128)
                nc.sync.dma_start(out_r[:, :, 0:D], C_f32[:])
                continue

            # ---- M3: P_C = E2^T @ [C|1] -> [64, 257] (col 256 = r2) ----
            pc = ps_mm.tile([LQ, D + 1], f32, tag="mm")
            for t in range(4):
                nc.tensor.matmul(
                    pc[:], E2[:, t, :], C_bf[:, t, :], start=(t == 0), stop=(t == 3)
                )
            rr2 = sb.tile([LQ, 1], f32, tag="rr2")
            nc.vector.reciprocal(rr2[:], pc[:, D : D + 1])
            C2_bf = sb.tile([LQ, D], bf16, tag="C2_bf")
            nc.vector.tensor_scalar_mul(C2_bf[:], pc[:, 0:D], rr2[:])

            if stage < 6:
                out_r = out_d[b].rearrange("(t p) dd -> p t dd", p=128)
                nc.sync.dma_start(out_r[:, :, 0:D], C_f32[:])
                continue

            # ---- M2: P_A[t] = E1 @ [Q|1] -> [128, 257] (col 256 = r1) ----
            rr1 = sb.tile([128, 4, 1], f32, tag="rr1")
            A_sb = sb.tile([128, 4, D], f32, tag="A_sb")
            pas = []
            for t in range(4):
                pa = ps_mm.tile([128, D + 1], f32, tag="mm")
                pas.append(pa)
                nc.tensor.matmul(
                    pa[:], E1_T[:, ts(t, 128)], Q_bf[:], start=True, stop=True
                )
                nc.vector.reciprocal(rr1[:, t, :], pa[:, D : D + 1])
                if t < 2:
                    nc.vector.tensor_scalar_mul(A_sb[:, t, :], pa[:, 0:D], rr1[:, t, :])
                else:
                    nc.scalar.mul(A_sb[:, t, :], pa[:, 0:D], rr1[:, t, :])

            if stage < 7:
                out_r = out_d[b].rearrange("(t p) dd -> p t dd", p=128)
                nc.sync.dma_start(out_r[:, :, 0:D], C_f32[:])
                nc.sync.dma_start(out_r[:, :, D : 2 * D], A_sb[:])
                continue

            # ---- M4: P_B[t] = E1 @ C2 -> Bm = P_B/r1 ----
            Bm_sb = sb.tile([128, 4, D], f32, tag="Bm_sb")
            for th in range(2):
                pb = ps_mm.tile([128, 2, D], f32, tag="mm")
                for h in range(2):
                    t = th * 2 + h
                    nc.tensor.matmul(
                        pb[:, h, :], E1_T[:, ts(t, 128)], C2_bf[:], start=True, stop=True
                    )
                    if t < 2:
                        nc.vector.tensor_scalar_mul(
                            Bm_sb[:, t, :], pb[:, h, :], rr1[:, t, :]
                        )
                    else:
                        nc.scalar.mul(Bm_sb[:, t, :], pb[:, h, :], rr1[:, t, :])

            # ---- outputs: [C | A | C*A | C*Bm] ----
            cA = sb.tile([128, 4, D], f32, tag="cA")
            nc.gpsimd.tensor_mul(cA[:], C_f32[:], A_sb[:])
            cBm = sb.tile([128, 4, D], f32, tag="cBm")
            nc.gpsimd.tensor_mul(cBm[:], C_f32[:], Bm_sb[:])

            out_r = out_d[b].rearrange("(t p) dd -> p t dd", p=128)
            nc.sync.dma_start(out_r[:, :, 0:D], C_f32[:])
            nc.sync.dma_start(out_r[:, :, D : 2 * D], A_sb[:])
            nc.sync.dma_start(out_r[:, :, 2 * D : 3 * D], cA[:])
            nc.sync.dma_start(out_r[:, :, 3 * D : 4 * D], cBm[:])

    nc.compile()
    return nc


def _get_nc():
    global _NC_CACHE
    if _NC_CACHE is None:
        import os

        _NC_CACHE = _build_nc(stage=int(os.environ.get("KERNEL_STAGE", "99")))
    return _NC_CACHE


def _make_in_maps(contex, question, W_weight):
    contex = np.asarray(contex, dtype=np.float32)
    question = np.asarray(question, dtype=np.float32)
    W_weight = np.asarray(W_weight, dtype=np.float32)
    in_maps = []
    for c in range(NCORES):
        sl = slice(c * BL, (c + 1) * BL)
        in_maps.append(
            {
                "contex": np.ascontiguousarray(contex[sl]),
                "question": np.ascontiguousarray(question[sl]),
                "W_weight": W_weight,
            }
        )
    return in_maps


def run_spmd(contex, question, W_weight, trace=False, tmpdir=None):
    """Returns (out [64,512,1024] f32, exec_time_ns or None)."""
    from concourse.bass_utils import run_bass_kernel_spmd

    nc = _get_nc()
    in_maps = _make_in_maps(contex, question, W_weight)
    res = run_bass_kernel_spmd(
        nc, in_maps, list(range(NCORES)), trace=trace, tmpdir=tmpdir
    )
    out = np.concatenate([res.results[c]["out"] for c in range(NCORES)], axis=0)
    return out, res.exec_time_ns


def kernel(contex, question, W_weight, W_bias=None, **_unused):
    # W_bias provably has no effect on the output (it is a constant shift
    # inside both softmaxes), so it is not shipped to the device.
    out, _ = run_spmd(contex, question, W_weight, trace=False)
    return out


# revision 24
# speedup vs baseline: 1.3239x; 1.3239x over previous
"""CQAttention (BiDAF-style context-query attention) on 8 TRN2 NeuronCores.

Full shapes: contex [64, 512, 256], question [64, 64, 256],
W_weight [1, 768], W_bias [1] -> out [64, 512, 1024].

Sharding: pure data-parallel over batch, 8 batches per core.

Math notes (per batch, C=[512,256], Q=[64,256], w=[wq|wc|wi]):
  S[i,j] = sum_d C[i,d]*wi[d]*Q[j,d] + C[i].wc + Q[j].wq + b
  S1 = softmax_j(S), S2 = softmax_i(S)
  - b drops out of both softmaxes; s_c drops out of S1; s_q drops out of S2.
  - E1 = exp(s_i + s_q[j]), r1[i] = sum_j E1;  S1 = E1/r1
  - E2 = exp(s_i + s_c[i]), r2[j] = sum_i E2;  S2 = E2/r2
  - A  = S1 @ Q = (E1 @ Q)/r1
  - Bm = (S1 @ S2^T) @ C = S1 @ (S2^T @ C) = (E1 @ C2)/r1, C2 = (E2^T @ C)/r2
  r1/r2 are obtained for free as ones-columns appended to the matmul rhs.
  out = [C | A | C*A | C*Bm]
"""

import numpy as np

B, LC, LQ, D = 64, 512, 64, 256
NCORES = 8
BL = B // NCORES  # batches per core

_NC_CACHE = None


def _build_nc(stage=99):
    """stage gates how much of the kernel is emitted (for hw bisection):
    1: loads+casts+output DMA of C only
    2: +PE transposes (+W broadcast matmul)
    3: +M1T+E1
    4: +M1'+E2
    5: +M3+C2
    6: +M2+A
    7+: full
    """
    import concourse.bass as bass
    import concourse.mybir as mybir
    from concourse import bacc
    from concourse import masks
    from concourse import tile
    from contextlib import ExitStack

    f32 = mybir.dt.float32
    bf16 = mybir.dt.bfloat16
    AF = mybir.ActivationFunctionType
    ALU = mybir.AluOpType
    ts = bass.ts

    nc = bacc.Bacc("TRN2", target_bir_lowering=False, debug=False)
    C_d = nc.dram_tensor("contex", [BL, LC, D], f32, kind="ExternalInput")
    Q_d = nc.dram_tensor("question", [BL, LQ, D], f32, kind="ExternalInput")
    W_d = nc.dram_tensor("W_weight", [1, 3 * D], f32, kind="ExternalInput")
    out_d = nc.dram_tensor("out", [BL, LC, 4 * D], f32, kind="ExternalOutput")

    with tile.TileContext(nc) as tc, ExitStack() as ctx:
        const = ctx.enter_context(tc.tile_pool(name="const", bufs=1))
        sb = ctx.enter_context(tc.tile_pool(name="sb", bufs=4))
        ps_tr = ctx.enter_context(tc.tile_pool(name="ps_tr", bufs=3, space="PSUM"))
        ps_si = ctx.enter_context(tc.tile_pool(name="ps_si", bufs=2, space="PSUM"))
        ps_mm = ctx.enter_context(tc.tile_pool(name="ps_mm", bufs=3, space="PSUM"))

        # ---- constants ----
        ident = const.tile([128, 128], bf16, tag="ident")
        masks.make_identity(nc, ident[:])

        W_sb = const.tile([1, 3 * D], f32, tag="W_sb")
        nc.sync.dma_start(W_sb[:], W_d[:])

        # wc as two [128,1] columns (k-th contraction tile), fp32
        wc_f32 = const.tile([128, 2, 1], f32, tag="wc_f32")
        nc.sync.dma_start(
            wc_f32[:], W_d[0, D : 2 * D].rearrange("(k p o) -> p k o", p=128, o=1)
        )

        if stage >= 2:
            # broadcast wq/wi rows to 64 partitions via K=1 matmul with ones.
            # Stage the rhs through DVE so the matmul waits on one engine only.
            W_sb2 = const.tile([1, 2, D], f32, tag="W_sb2")
            nc.vector.tensor_copy(W_sb2[:, 0, :], W_sb[0:1, 0:D])
            nc.vector.tensor_copy(W_sb2[:, 1, :], W_sb[0:1, 2 * D : 3 * D])
            ones_row = const.tile([1, LQ], f32, tag="ones_row")
            nc.vector.memset(ones_row[:], 1.0)
            wb_ps = ps_si.tile([LQ, 2, D], f32, tag="si")
            nc.tensor.matmul(wb_ps[:], ones_row[:], W_sb2[:], start=True, stop=True)
            wqi = const.tile([LQ, 2, D], f32, tag="wqi")
            nc.scalar.copy(wqi[:], wb_ps[:])
            wq_b = wqi[:, 0, :]  # [64, 256] rows = wq
            wi_b = wqi[:, 1, :]  # [64, 256] rows = wi

        for b in range(BL):
            # ---- loads (input DMAs on the ACT HWDGE ring so the big output
            # stores on the SP ring can't head-of-line-block them) ----
            C_f32 = sb.tile([128, 4, D], f32, tag="C_f32")
            nc.scalar.dma_start(C_f32[:], C_d[b].rearrange("(t p) d -> p t d", p=128))
            Q_f32 = sb.tile([LQ, D], f32, tag="Q_f32")
            nc.scalar.dma_start(Q_f32[:], Q_d[b])

            # ---- bf16 casts / augmented operands ----
            # C_bf: [128, 4, 257]: cols 0:256 = C (bf16), col 256 = 1.0
            C_bf = sb.tile([128, 4, D + 1], bf16, tag="C_bf")
            nc.scalar.copy(C_bf[:, :, 0:D], C_f32[:])
            nc.gpsimd.memset(C_bf[:, :, D : D + 1], 1.0)

            # Q_bf: [64, 257]: cols 0:256 = Q (bf16), col 256 = 1.0
            Q_bf = sb.tile([LQ, D + 1], bf16, tag="Q_bf")
            nc.vector.tensor_copy(Q_bf[:, 0:D], Q_f32[:])
            nc.gpsimd.memset(Q_bf[:, D : D + 1], 1.0)

            if stage < 2 or stage == 21:
                out_r = out_d[b].rearrange("(t p) dd -> p t dd", p=128)
                nc.sync.dma_start(out_r[:, :, 0:D], C_f32[:])
                continue

            # Q' = Q * wi (bf16), s_q = rowsum(Q * wq) (f32)
            # (tensor_tensor_reduce crashes the exec unit on this runtime —
            # use separate mul + reduce instead)
            QP_bf = sb.tile([LQ, D], bf16, tag="QP_bf")
            nc.gpsimd.tensor_mul(QP_bf[:], Q_f32[:], wi_b)
            scr = sb.tile([LQ, D], f32, tag="scr")
            s_q = sb.tile([LQ, 1], f32, tag="s_q")
            nc.vector.tensor_mul(scr[:], Q_f32[:], wq_b)
            nc.vector.reduce_sum(s_q[:], scr[:], axis=mybir.AxisListType.X)

            if stage in (22, 221, 222):
                out_r = out_d[b].rearrange("(t p) dd -> p t dd", p=128)
                nc.sync.dma_start(out_r[:, :, 0:D], C_f32[:])
                continue

            # ---- transposes (PE) ----
            # tq: Q'^T -> [128, 2*64]; QW[k] = [Q'^T_k | wc_k]  [128, 2, 65]
            tq = ps_tr.tile([128, 128], bf16, tag="tr")
            for k in range(2):
                nc.tensor.transpose(
                    tq[:, ts(k, 64)], QP_bf[:, ts(k, 128)], ident[0:LQ, 0:LQ]
                )
            QW = sb.tile([128, 2, 65], bf16, tag="QW")
            nc.vector.tensor_copy(
                QW[:, :, 0:64], tq[:].rearrange("p (k j) -> p k j", k=2)
            )
            nc.vector.tensor_copy(QW[:, :, 64:65], wc_f32[:])

            if stage == 23:
                out_r = out_d[b].rearrange("(t p) dd -> p t dd", p=128)
                nc.sync.dma_start(out_r[:, :, 0:D], C_f32[:])
                continue

            # tc: C^T -> CT [128, 2, 512] (k = d-tile, free = i)
            tcp = ps_tr.tile([128, 2, 512], bf16, tag="tr")
            for t in range(4):
                for k in range(2):
                    nc.tensor.transpose(
                        tcp[:, k, ts(t, 128)],
                        C_bf[:, t, ts(k, 128)],
                        ident[:],
                    )
            CT = sb.tile([128, 2, 512], bf16, tag="CT")
            nc.vector.tensor_copy(CT[:, 0, :], tcp[:, 0, :])
            nc.scalar.copy(CT[:, 1, :], tcp[:, 1, :])

            if stage < 3:
                out_r = out_d[b].rearrange("(t p) dd -> p t dd", p=128)
                nc.sync.dma_start(out_r[:, :, 0:D], C_f32[:])
                continue

            # ---- M1T: s_i^T [65, 512] (row 64 = s_c, unused) ----
            si_T = ps_si.tile([65, 512], f32, tag="si")
            for k in range(2):
                nc.tensor.matmul(
                    si_T[:], QW[:, k, :], CT[:, k, :], start=(k == 0), stop=(k == 1)
                )
            # E1_T = exp(s_i^T + s_q) (bf16)  [64, 512]
            E1_T = sb.tile([LQ, 512], bf16, tag="E1_T")
            nc.scalar.activation(E1_T[:], si_T[0:LQ, :], AF.Exp, bias=s_q[:])

            if stage < 4:
                out_r = out_d[b].rearrange("(t p) dd -> p t dd", p=128)
                nc.sync.dma_start(out_r[:, :, 0:D], C_f32[:])
                continue

            # ---- M1': s_i natural [128, 4, 65] (col 64 = s_c) ----
            si_n = ps_si.tile([128, 4, 65], f32, tag="si")
            for t in range(4):
                for k in range(2):
                    nc.tensor.matmul(
                        si_n[:, t, :],
                        CT[:, k, ts(t, 128)],
                        QW[:, k, :],
                        start=(k == 0),
                        stop=(k == 1),
                    )
            sc = sb.tile([128, 4, 1], f32, tag="sc")
            nc.vector.tensor_copy(sc[:], si_n[:, :, 64:65])
            # E2 = exp(s_i + s_c) (bf16)  [128, 4, 64]
            E2 = sb.tile([128, 4, 64], bf16, tag="E2")
            for t in range(4):
                nc.scalar.activation(
                    E2[:, t, :], si_n[:, t, 0:64], AF.Exp, bias=sc[:, t, :]
                )

            if stage < 5:
                out_r = out_d[b].rearrange("(t p) dd -> p t dd", p=128)
                nc.sync.dma_start(out_r[:, :, 0:D], C_f32[:])
                continue

            # ---- M3: P_C = E2^T @ [C|1] -> [64, 257] (col 256 = r2) ----
            pc = ps_mm.tile([LQ, D + 1], f32, tag="mm")
            for t in range(4):
                nc.tensor.matmul(
                    pc[:], E2[:, t, :], C_bf[:, t, :], start=(t == 0), stop=(t == 3)
                )
            rr2 = sb.tile([LQ, 1], f32, tag="rr2")
            nc.vector.reciprocal(rr2[:], pc[:, D : D + 1])
            C2_bf = sb.tile([LQ, D], bf16, tag="C2_bf")
            nc.vector.tensor_scalar_mul(C2_bf[:], pc[:, 0:D], rr2[:])

            if stage < 6:
                out_r = out_d[b].rearrange("(t p) dd -> p t dd", p=128)
                nc.sync.dma_start(out_r[:, :, 0:D], C_f32[:])
                continue

            # ---- M2: P_A[t] = E1 @ [Q|1] -> [128, 257] (col 256 = r1) ----
            rr1 = sb.tile([128, 4, 1], f32, tag="rr1")
            A_sb = sb.tile([128, 4, D], f32, tag="A_sb")
            pas = []
            for t in range(4):
                pa = ps_mm.tile([128, D + 1], f32, tag="mm")
                pas.append(pa)
                nc.tensor.matmul(
                    pa[:], E1_T[:, ts(t, 128)], Q_bf[:], start=True, stop=True
                )
                nc.vector.reciprocal(rr1[:, t, :], pa[:, D : D + 1])
                if t < 2:
                    nc.vector.tensor_scalar_mul(A_sb[:, t, :], pa[:, 0:D], rr1[:, t, :])
                else:
                    nc.scalar.mul(A_sb[:, t, :], pa[:, 0:D], rr1[:, t, :])

            if stage < 7:
                out_r = out_d[b].rearrange("(t p) dd -> p t dd", p=128)
                nc.sync.dma_start(out_r[:, :, 0:D], C_f32[:])
                nc.sync.dma_start(out_r[:, :, D : 2 * D], A_sb[:])
                continue

            # ---- M4: P_B[t] = E1 @ C2 -> Bm = P_B/r1 ----
            Bm_sb = sb.tile([128, 4, D], f32, tag="Bm_sb")
            for th in range(2):
                pb = ps_mm.tile([128, 2, D], f32, tag="mm")
                for h in range(2):
                    t = th * 2 + h
                    nc.tensor.matmul(
                        pb[:, h, :], E1_T[:, ts(t, 128)], C2_bf[:], start=True, stop=True
                    )
                    if t < 2:
                        nc.vector.tensor_scalar_mul(
                            Bm_sb[:, t, :], pb[:, h, :], rr1[:, t, :]
                        )
                    else:
                        nc.scalar.mul(Bm_sb[:, t, :], pb[:, h, :], rr1[:, t, :])

            # ---- outputs: [C | A | C*A | C*Bm] ----
            cA = sb.tile([128, 4, D], f32, tag="cA")
            nc.vector.tensor_mul(cA[:], C_f32[:], A_sb[:])
            cBm = sb.tile([128, 4, D], f32, tag="cBm")
            nc.gpsimd.tensor_mul(cBm[:], C_f32[:], Bm_sb[:])

            out_r = out_d[b].rearrange("(t p) dd -> p t dd", p=128)
            nc.sync.dma_start(out_r[:, :, 0:D], C_f32[:])
            nc.sync.dma_start(out_r[:, :, D : 2 * D], A_sb[:])
            nc.sync.dma_start(out_r[:, :, 2 * D : 3 * D], cA[:])
            nc.sync.dma_start(out_r[:, :, 3 * D : 4 * D], cBm[:])

    nc.compile()
    return nc


def _get_nc():
    global _NC_CACHE
    if _NC_CACHE is None:
        import os

        _NC_CACHE = _build_nc(stage=int(os.environ.get("KERNEL_STAGE", "99")))
    return _NC_CACHE


def _make_in_maps(contex, question, W_weight):
    contex = np.asarray(contex, dtype=np.float32)
    question = np.asarray(question, dtype=np.float32)
    W_weight = np.asarray(W_weight, dtype=np.float32)
    in_maps = []
    for c in range(NCORES):
        sl = slice(c * BL, (c + 1) * BL)
        in_maps.append(
            {
                "contex": np.ascontiguousarray(contex[sl]),
                "question": np.ascontiguousarray(question[sl]),
                "W_weight": W_weight,
            }
        )
    return in_maps


def run_spmd(contex, question, W_weight, trace=False, tmpdir=None):
    """Returns (out [64,512,1024] f32, exec_time_ns or None)."""
    from concourse.bass_utils import run_bass_kernel_spmd

    nc = _get_nc()
    in_maps = _make_in_maps(contex, question, W_weight)
    res = run_bass_kernel_spmd(
        nc, in_maps, list(range(NCORES)), trace=trace, tmpdir=tmpdir
    )
    out = np.concatenate([res.results[c]["out"] for c in range(NCORES)], axis=0)
    return out, res.exec_time_ns


def kernel(contex, question, W_weight, W_bias=None, **_unused):
    # W_bias provably has no effect on the output (it is a constant shift
    # inside both softmaxes), so it is not shipped to the device.
    out, _ = run_spmd(contex, question, W_weight, trace=False)
    return out
